# revision 31
# baseline (speedup 1.0000x reference)
"""Causal self-attention (GQA + RoPE) Trainium2 Bass kernel, 8-core SPMD.

Problem shapes (hardcoded): B=2, S=2048, D=1024, NH=16 q-heads, KVH=4
kv-heads, HD=64, RoPE base 10000, fp32 I/O.

Sharding (batch x kv-group): core c -> batch b = c//4, kv-group g = c%4.
Each kv-group owns one kv head and its 4 q heads (GQA repeat=4), so the
whole causal attention for those heads is local to the core. Each core
computes the partial output projection y_g @ Wo[g-block]; the host sums
the 4 partials per batch ("gather/unshard").

Per-core device kernel:
  inputs : xT [1024, 2048] (x[b] transposed, bf16), wq [1024, 256],
           wkv [1024, 128] (Wk_g ++ Wv_g), wo [256, 1024]  (all bf16)
  output : out [2048, 1024] bf16 partial

Everything on-chip is kept transposed ([head_dim, seq]) so QK^T and PV
need no transposes. Softmax denominators ride along the PV accumulation
via an augmented-ones column in v; the two heads of a PSUM pair use
mirrored augmentations ([v|1] vs [1|v]) so each head's divided output
lands lane-aligned with its slot in the stacked Y tile (no shift DMA).
Causality: fully-masked key chunks are skipped, scores matmuls are
trimmed to the valid query range on the last diagonal pair, and the
128x128 triangular blocks are masked by a second accumulating matmul
(-1e9 * strict-lower-triangle) into the scores PSUM group - no vector
op on the exp critical path. The per-query reciprocal of the softmax
denominator is broadcast across partitions with a GPSIMD
partition_broadcast, keeping the tensor engine stream dense.
"""
import numpy as np
from contextlib import ExitStack

import ml_dtypes

import concourse.bass as bass
import concourse.tile as tile
from concourse import bacc, mybir
from concourse.bass_utils import run_bass_kernel_spmd

F32 = mybir.dt.float32
BF16 = mybir.dt.bfloat16
AF = mybir.ActivationFunctionType

B, S, D = 2, 2048, 1024
NH, KVH, HD = 16, 4, 64
N_CORES = 8
SCALE = HD ** -0.5  # 0.125
NEG = -1.0e9

_CACHE = {}


def _rope_tables():
    half = HD // 2
    inv_freq = (1.0 / (10000.0 ** (np.arange(half, dtype=np.float32) / half))
                ).astype(np.float32)
    t = np.arange(S, dtype=np.float32)
    freqs = np.outer(t, inv_freq).astype(np.float32)      # [S, 32]
    emb = np.concatenate([freqs, freqs], axis=1)          # [S, 64]
    cos_T = np.cos(emb).T.astype(np.float32)              # [64, S]
    sin_T = np.sin(emb).T.astype(np.float32)
    sin_n = sin_T.copy()
    sin_n[:half] *= -1.0                                  # sign for rotate_half
    cos4 = np.tile(cos_T, (2, 1))                         # [128, S] (2 heads)
    # halves swapped within each 64-row block: row b+32+i holds sin_n[b+i]
    # so the shifted-read DVE multiply uses equal input base partitions.
    sin_r = np.concatenate([sin_n[half:], sin_n[:half]], axis=0)
    sin4r = np.tile(sin_r, (2, 1))
    return cos4, sin4r


def _build_kernel():
    nc = bacc.Bacc("TRN2", target_bir_lowering=False, debug=False,
                   num_devices=N_CORES)

    xT_ap = nc.dram_tensor("xT", [D, S], BF16, kind="ExternalInput").ap()
    wq_ap = nc.dram_tensor("wq", [D, 256], BF16, kind="ExternalInput").ap()
    wkv_ap = nc.dram_tensor("wkv", [D, 128], BF16, kind="ExternalInput").ap()
    wo_ap = nc.dram_tensor("wo", [256, D], BF16, kind="ExternalInput").ap()
    out_ap = nc.dram_tensor("out", [S, D], BF16, kind="ExternalOutput").ap()

    bf = ml_dtypes.bfloat16
    cos4_np, sin4_np = _rope_tables()
    cos4_d = nc.inline_tensor(cos4_np.astype(bf), name="cos4").ap()
    sin4_d = nc.inline_tensor(sin4_np.astype(bf), name="sin4").ap()
    # strict lower triangle ones [k, q]: 1 where q < k (to be masked)
    tri_np = (np.arange(128)[None, :] < np.arange(128)[:, None]
              ).astype(np.float32)
    tri_d = nc.inline_tensor(tri_np.astype(bf), name="tri").ap()
    negid_np = (NEG * np.eye(128)).astype(np.float32)
    negid_d = nc.inline_tensor(negid_np.astype(bf), name="negid").ap()
    ones16_d = nc.inline_tensor(np.ones((128, 16), bf), name="ones16").ap()
    ones64_d = nc.inline_tensor(np.ones((1, 64), np.float32),
                                name="ones64").ap()
    id_np = np.zeros((128, 64), np.float32)
    id_np[64:128] = np.eye(64, dtype=np.float32)
    id_d = nc.inline_tensor(id_np.astype(bf), name="id64").ap()

    with tile.TileContext(nc) as tc, ExitStack() as top:
        # ---- constants + persistent sbuf tiles -------------------------
        const = top.enter_context(tc.tile_pool(name="const", bufs=1))
        cos4 = const.tile([128, S], BF16, tag="cos4")
        sin4 = const.tile([128, S], BF16, tag="sin4")
        tri = const.tile([128, 128], BF16, tag="tri")
        negid = const.tile([128, 128], BF16, tag="negid")
        id64 = const.tile([128, 64], BF16, tag="id64")
        ones_r = const.tile([1, 64], mybir.dt.float32r, tag="ones_r")
        # consts ride the scalar-engine HWDGE queue so they don't delay the
        # critical weights+x stream on the gpsimd SWDGE queue.
        nc.scalar.dma_start(id64[:], id_d[:])
        nc.scalar.dma_start(cos4[:], cos4_d[:])
        nc.scalar.dma_start(sin4[:], sin4_d[:])
        nc.scalar.dma_start(tri[:], tri_d[:])
        nc.scalar.dma_start(negid[:], negid_d[:])
        nc.gpsimd.dma_start(ones_r[:], ones64_d[:])

        wpool = top.enter_context(tc.tile_pool(name="w", bufs=1))
        wq_sb = wpool.tile([128, 8 * 256], BF16, tag="wq")
        wkv_sb = wpool.tile([128, 8 * 128], BF16, tag="wkv")
        wo_sb = wpool.tile([128, 2 * 1024], BF16, tag="wo")
        # qkv weights ride the sync HWDGE queue, in parallel with the x
        # chunks on the gpsimd SWDGE queue.
        nc.sync.dma_start(wkv_sb[:].rearrange("p (kc m) -> p kc m", kc=8),
                          wkv_ap.rearrange("(kc p) m -> p kc m", p=128))
        nc.sync.dma_start(wq_sb[:].rearrange("p (kc m) -> p kc m", kc=8),
                          wq_ap.rearrange("(kc p) m -> p kc m", p=128))

        # q'/k/v results live through the whole kernel
        act = top.enter_context(tc.tile_pool(name="acts", bufs=1))
        qp = [act.tile([128, S], BF16, tag=f"qp{i}", name=f"qp{i}")
              for i in range(2)]
        kk = act.tile([128, S], BF16, tag="kk")
        # v augmented with a ones column: PV drops the softmax denominator
        # into row 64 of each head's PSUM accumulator.
        v_all = act.tile([128, 16 * 80], BF16, tag="v_all")
        Y = [act.tile([128, S], BF16, tag=f"Y{i}", name=f"Y{i}")
             for i in range(2)]

        # ---- phase 1: projections, RoPE, v prep ------------------------
        with ExitStack() as ph1:
            raw = ph1.enter_context(tc.tile_pool(name="raw", bufs=1))
            qraw = [raw.tile([128, S], BF16, tag=f"qraw{i}", name=f"qraw{i}")
                    for i in range(2)]
            kvraw = raw.tile([128, S], BF16, tag="kvraw")
            qsh = [raw.tile([128, S], BF16, tag=f"qsh{i}", name=f"qsh{i}")
                   for i in range(2)]
            ksh = raw.tile([64, S], BF16, tag="ksh")

            phx = ph1.enter_context(ExitStack())
            xpool = phx.enter_context(tc.tile_pool(name="xT", bufs=1))
            xT = []
            for kc in range(8):
                t = xpool.tile([128, S], BF16, tag=f"x{kc}", name=f"x{kc}")
                nc.gpsimd.dma_start(t[:], xT_ap[kc * 128:(kc + 1) * 128, :])
                xT.append(t)
            # wo is not needed until the first out-projection (~mid-kernel);
            # issue it behind the x chunks.
            nc.gpsimd.dma_start(wo_sb[:].rearrange("p (c n) -> p c n", c=2),
                                wo_ap.rearrange("(c p) n -> p c n", p=128))

            pps = phx.enter_context(tc.tile_pool(name="pj", bufs=8,
                                                 space="PSUM"))
            with nc.named_scope("proj"):
                # kc-outer: each xT chunk is consumed right after its DMA
                # lands; kv + q-mt0 accumulate in pass 1 (8 banks), q-mt1
                # in pass 2.
                kv_ps = [pps.tile([128, 512], F32, tag="pj", name=f"kvps{nt}")
                         for nt in range(4)]
                q0_ps = [pps.tile([128, 512], F32, tag="pj", name=f"q0ps{nt}")
                         for nt in range(4)]
                for kc in range(8):
                    for nt in range(4):
                        nc.tensor.matmul(
                            kv_ps[nt][:], wkv_sb[:, kc * 128:(kc + 1) * 128],
                            xT[kc][:, nt * 512:(nt + 1) * 512],
                            start=(kc == 0), stop=(kc == 7))
                        nc.tensor.matmul(
                            q0_ps[nt][:], wq_sb[:, kc * 256:kc * 256 + 128],
                            xT[kc][:, nt * 512:(nt + 1) * 512],
                            start=(kc == 0), stop=(kc == 7))
                # kv copies on the scalar engine (idle in phase 1); they
                # gate the q1 bank reuse and the v transposes.
                for nt in range(4):
                    nc.scalar.copy(kvraw[:, nt * 512:(nt + 1) * 512],
                                   kv_ps[nt][:])
                q1_ps = [pps.tile([128, 512], F32, tag="pj", name=f"q1ps{nt}")
                         for nt in range(4)]
                for kc in range(8):
                    for nt in range(4):
                        nc.tensor.matmul(
                            q1_ps[nt][:], wq_sb[:, kc * 256 + 128:kc * 256 + 256],
                            xT[kc][:, nt * 512:(nt + 1) * 512],
                            start=(kc == 0), stop=(kc == 7))
                for nt in range(4):
                    nc.vector.tensor_copy(qraw[0][:, nt * 512:(nt + 1) * 512],
                                          q0_ps[nt][:])
                for nt in range(4):
                    if nt % 2 == 0:
                        nc.scalar.copy(qraw[1][:, nt * 512:(nt + 1) * 512],
                                       q1_ps[nt][:])
                    else:
                        nc.vector.tensor_copy(
                            qraw[1][:, nt * 512:(nt + 1) * 512], q1_ps[nt][:])
            phx.close()

            with nc.named_scope("rope"):
                # rotate_half via partition-shifted DVE writes; the sin table
                # has its 32-row halves pre-swapped so both SBUF inputs share
                # a base partition (DVE requires equal SB-input bases; only
                # the output base may differ).
                nc.vector.tensor_mul(ksh[0:32, :], kvraw[32:64, :],
                                     sin4[32:64, :])
                nc.vector.tensor_mul(ksh[32:64, :], kvraw[0:32, :],
                                     sin4[0:32, :])
                nc.vector.tensor_mul(kk[0:64, :], kvraw[0:64, :], cos4[0:64, :])
                nc.vector.tensor_add(kk[0:64, :], kk[0:64, :], ksh[0:64, :])
                nc.vector.tensor_copy(kk[64:128, :], kk[0:64, :])
                for i in range(2):
                    for h in range(2):
                        b = h * 64
                        nc.vector.tensor_mul(qsh[i][b:b + 32, :],
                                             qraw[i][b + 32:b + 64, :],
                                             sin4[b + 32:b + 64, :])
                        nc.vector.tensor_mul(qsh[i][b + 32:b + 64, :],
                                             qraw[i][b:b + 32, :],
                                             sin4[b:b + 32, :])
                    nc.vector.tensor_mul(qp[i][:], qraw[i][:], cos4[:])
                    nc.vector.tensor_add(qp[i][:], qp[i][:], qsh[i][:])

            with nc.named_scope("vprep"), ExitStack() as ph3:
                vps = ph3.enter_context(tc.tile_pool(name="vt", bufs=2,
                                                     space="PSUM"))
                ones_cols = v_all[:].rearrange("p (s c) -> p s c", c=80)[:, :, 64]
                nc.scalar.dma_start(ones_cols, ones16_d[:])
                for st in range(16):
                    tp = vps.tile([128, 64], BF16)
                    nc.tensor.transpose(
                        tp[:], kvraw[64:128, st * 128:(st + 1) * 128],
                        id64[64:128, :])
                    nc.vector.tensor_copy(v_all[:, st * 80:st * 80 + 64],
                                          tp[:])

        # ---- attention + interleaved out-projection --------------------
        apool = top.enter_context(tc.tile_pool(name="at", bufs=2, space="PSUM"))
        epool = top.enter_context(tc.tile_pool(name="ex", bufs=3))
        dpool = top.enter_context(tc.tile_pool(name="div", bufs=2))
        osb = top.enter_context(tc.tile_pool(name="osb", bufs=4))
        oev = [0]

        def emit_outproj(qt):
            with nc.named_scope("outproj"):
                for st in range(4 * qt, 4 * qt + 4):
                    for nt in range(2):
                        po = apool.tile([128, 512], F32, tag="yo", bufs=4,
                                        name=f"po{st}{nt}")
                        for cc in range(2):
                            nc.tensor.matmul(
                                po[:],
                                Y[cc][:, st * 128:(st + 1) * 128],
                                wo_sb[:, cc * 1024 + nt * 512:
                                      cc * 1024 + (nt + 1) * 512],
                                start=(cc == 0), stop=(cc == 1))
                        ot = osb.tile([128, 512], BF16, tag="ot")
                        nc.vector.tensor_copy(ot[:], po[:])
                        oev[0] += 1
                        nc.sync.dma_start(
                            out_ap[st * 128:(st + 1) * 128,
                                   nt * 512:(nt + 1) * 512],
                            ot[:])

        with nc.named_scope("attn"):
            # descending qt: the dense q-tiles run first, keeping the PE's
            # HAM activity monitor at full clock through the early phase;
            # the sparse tail interleaves with out-projection filler.
            for qt in (3, 2, 1, 0):
                for pair in range(2):
                    nkc = 4 * qt + 4
                    yt = [apool.tile([128, 512], F32, tag="yo", bufs=4,
                                     name=f"y{pair}{qt}{_h}") for _h in range(2)]
                    for G in range(nkc // 2):
                        for hl in range(2):
                            hb = hl * 64
                            sc = apool.tile([128, 1024], F32, tag="sc", bufs=2,
                                            name=f"sc{pair}{qt}{G}{hl}")
                            regions = []
                            for ci in range(2):
                                kc = 2 * G + ci
                                j = kc - 4 * qt
                                off = j * 128 if j >= 0 else 0
                                trim = off if j >= 2 else 0
                                c0 = ci * 512
                                nc.tensor.matmul(
                                    sc[:, c0 + trim:c0 + 512],
                                    kk[hb:hb + 64, kc * 128:(kc + 1) * 128],
                                    qp[pair][hb:hb + 64,
                                             qt * 512 + trim:(qt + 1) * 512],
                                    start=True, stop=(j < 0))
                                if j >= 0:
                                    nc.tensor.matmul(
                                        sc[:, c0 + off:c0 + off + 128],
                                        negid[:, :], tri[:, :],
                                        start=False, stop=True)
                                regions.append((c0 + trim, c0 + 512))
                            ex = epool.tile([128, 1024], BF16, tag="ex",
                                            name=f"ex{pair}{qt}{G}{hl}")
                            if regions[0][0] == 0 and regions[1][0] == 512:
                                nc.scalar.activation(ex[:, 0:1024],
                                                     sc[:, 0:1024],
                                                     AF.Exp, scale=SCALE)
                            else:
                                for (a, b) in regions:
                                    nc.scalar.activation(ex[:, a:b],
                                                         sc[:, a:b],
                                                         AF.Exp, scale=SCALE)
                            for ci in range(2):
                                kc = 2 * G + ci
                                j = kc - 4 * qt
                                off = j * 128 if j >= 0 else 0
                                nc.tensor.matmul(
                                    yt[hl][0:65, off:512],
                                    v_all[:, kc * 80:kc * 80 + 65],
                                    ex[:, ci * 512 + off:(ci + 1) * 512],
                                    start=(kc == 0), stop=(kc == nkc - 1))
                    # division (baseline flow): copy y+den to SBUF, recip,
                    # f32r cast, K=1 broadcast matmul, multiply, cast-DMA
                    # into the stacked bf16 Y tile.
                    ysb = dpool.tile([64, 1024], F32, tag="ysb")
                    dn = dpool.tile([1, 1024], F32, tag="dn")
                    nc.vector.tensor_copy(ysb[:, 0:512], yt[0][0:64, :])
                    nc.vector.tensor_copy(ysb[:, 512:1024], yt[1][0:64, :])
                    nc.scalar.copy(dn[:, 0:512], yt[0][64:65, :])
                    nc.scalar.copy(dn[:, 512:1024], yt[1][64:65, :])
                    recf = dpool.tile([1, 1024], F32, tag="recf")
                    nc.vector.reciprocal_approx_fast(recf[:], dn[:])
                    recr = dpool.tile([1, 1024], mybir.dt.float32r, tag="recr")
                    nc.vector.tensor_copy(recr[:], recf[:])
                    for hl in range(2):
                        bc_ps = apool.tile([128, 512], F32, tag="yo", bufs=4,
                                           name=f"bc{pair}{qt}{hl}")
                        nc.tensor.matmul(bc_ps[0:64, :], ones_r[:],
                                         recr[0:1, hl * 512:(hl + 1) * 512],
                                         start=True, stop=True)
                        # one PSUM operand + partition-shifted write into
                        # the stacked bf16 Y tile
                        nc.vector.tensor_mul(
                            Y[pair][hl * 64:hl * 64 + 64,
                                    qt * 512:(qt + 1) * 512],
                            ysb[:, hl * 512:(hl + 1) * 512], bc_ps[0:64, :])
                    # out-projection of the previously processed q-tile,
                    # emitted here so the PE fills the gap while this
                    # q-tile's exps drain.
                    if pair == 0 and qt <= 2:
                        emit_outproj(qt + 1)
            emit_outproj(0)

    nc.compile()
    return nc


def _shard_inputs(x, Wq, Wk, Wv, Wo):
    bf = ml_dtypes.bfloat16
    in_maps = []
    for c in range(N_CORES):
        b, g = divmod(c, 4)
        in_maps.append({
            "xT": np.ascontiguousarray(x[b].T).astype(bf),
            "wq": np.ascontiguousarray(
                Wq[:, g * 256:(g + 1) * 256]).astype(bf),
            "wkv": np.ascontiguousarray(np.concatenate(
                [Wk[:, g * 64:(g + 1) * 64], Wv[:, g * 64:(g + 1) * 64]],
                axis=1)).astype(bf),
            "wo": np.ascontiguousarray(
                Wo[g * 256:(g + 1) * 256, :]).astype(bf),
        })
    return in_maps


def kernel(x, Wq, Wk, Wv, Wo):
    x = np.asarray(x, dtype=np.float32)
    Wq = np.asarray(Wq, dtype=np.float32)
    Wk = np.asarray(Wk, dtype=np.float32)
    Wv = np.asarray(Wv, dtype=np.float32)
    Wo = np.asarray(Wo, dtype=np.float32)
    assert x.shape == (B, S, D), x.shape

    if "nc" not in _CACHE:
        _CACHE["nc"] = _build_kernel()
    nc = _CACHE["nc"]

    in_maps = _shard_inputs(x, Wq, Wk, Wv, Wo)
    res = run_bass_kernel_spmd(nc, in_maps, list(range(N_CORES)))

    out = np.zeros((B, S, D), dtype=np.float32)
    for c in range(N_CORES):
        out[c // 4] += np.asarray(res.results[c]["out"]).astype(np.float32)
    return out


# revision 37
# speedup vs baseline: 1.0368x; 1.0368x over previous
"""Causal self-attention (GQA + RoPE) Trainium2 Bass kernel, 8-core SPMD.

Problem shapes (hardcoded): B=2, S=2048, D=1024, NH=16 q-heads, KVH=4
kv-heads, HD=64, RoPE base 10000, fp32 I/O.

Sharding (batch x kv-group): core c -> batch b = c//4, kv-group g = c%4.
Each kv-group owns one kv head and its 4 q heads (GQA repeat=4), so the
whole causal attention for those heads is local to the core. Each core
computes the partial output projection y_g @ Wo[g-block]; the host sums
the 4 partials per batch ("gather/unshard").

Per-core device kernel:
  inputs : xT [1024, 2048] (x[b] transposed, bf16), wq [1024, 256],
           wkv [1024, 128] (Wk_g ++ Wv_g), wo [256, 1024]  (all bf16)
  output : out [2048, 1024] bf16 partial

Everything on-chip is kept transposed ([head_dim, seq]) so QK^T and PV
need no transposes. Softmax denominators ride along the PV accumulation
via an augmented-ones column in v; the two heads of a PSUM pair use
mirrored augmentations ([v|1] vs [1|v]) so each head's divided output
lands lane-aligned with its slot in the stacked Y tile (no shift DMA).
Causality: fully-masked key chunks are skipped, scores matmuls are
trimmed to the valid query range on the last diagonal pair, and the
128x128 triangular blocks are masked by a second accumulating matmul
(-1e9 * strict-lower-triangle) into the scores PSUM group - no vector
op on the exp critical path. The per-query reciprocal of the softmax
denominator is broadcast across partitions with a GPSIMD
partition_broadcast, keeping the tensor engine stream dense.
"""
import numpy as np
from contextlib import ExitStack

import ml_dtypes

import concourse.bass as bass
import concourse.tile as tile
from concourse import bacc, mybir
from concourse.bass_utils import run_bass_kernel_spmd

F32 = mybir.dt.float32
BF16 = mybir.dt.bfloat16
AF = mybir.ActivationFunctionType

B, S, D = 2, 2048, 1024
NH, KVH, HD = 16, 4, 64
N_CORES = 8
SCALE = HD ** -0.5  # 0.125
NEG = -1.0e9

_CACHE = {}


def _rope_tables():
    half = HD // 2
    inv_freq = (1.0 / (10000.0 ** (np.arange(half, dtype=np.float32) / half))
                ).astype(np.float32)
    t = np.arange(S, dtype=np.float32)
    freqs = np.outer(t, inv_freq).astype(np.float32)      # [S, 32]
    emb = np.concatenate([freqs, freqs], axis=1)          # [S, 64]
    cos_T = np.cos(emb).T.astype(np.float32)              # [64, S]
    sin_T = np.sin(emb).T.astype(np.float32)
    sin_n = sin_T.copy()
    sin_n[:half] *= -1.0                                  # sign for rotate_half
    cos4 = np.tile(cos_T, (2, 1))                         # [128, S] (2 heads)
    # halves swapped within each 64-row block: row b+32+i holds sin_n[b+i]
    # so the shifted-read DVE multiply uses equal input base partitions.
    sin_r = np.concatenate([sin_n[half:], sin_n[:half]], axis=0)
    sin4r = np.tile(sin_r, (2, 1))
    return cos4, sin4r


def _build_kernel():
    nc = bacc.Bacc("TRN2", target_bir_lowering=False, debug=False,
                   num_devices=N_CORES)

    xT_ap = nc.dram_tensor("xT", [D, S], BF16, kind="ExternalInput").ap()
    wq_ap = nc.dram_tensor("wq", [D, 256], BF16, kind="ExternalInput").ap()
    wkv_ap = nc.dram_tensor("wkv", [D, 128], BF16, kind="ExternalInput").ap()
    wo_ap = nc.dram_tensor("wo", [256, D], BF16, kind="ExternalInput").ap()
    out_ap = nc.dram_tensor("out", [S, D], BF16, kind="ExternalOutput").ap()

    bf = ml_dtypes.bfloat16
    cos4_np, sin4_np = _rope_tables()
    cos4_d = nc.inline_tensor(cos4_np.astype(bf), name="cos4").ap()
    sin4_d = nc.inline_tensor(sin4_np.astype(bf), name="sin4").ap()
    # keep-mask [k, q]: 1 where q >= k (causal keep)
    tri_np = (np.arange(128)[None, :] >= np.arange(128)[:, None]
              ).astype(np.float32)
    tri_d = nc.inline_tensor(tri_np.astype(bf), name="tri").ap()
    ones16_d = nc.inline_tensor(np.ones((128, 16), bf), name="ones16").ap()
    ones64_d = nc.inline_tensor(np.ones((1, 64), np.float32),
                                name="ones64").ap()
    id_np = np.zeros((128, 64), np.float32)
    id_np[64:128] = np.eye(64, dtype=np.float32)
    id_d = nc.inline_tensor(id_np.astype(bf), name="id64").ap()

    with tile.TileContext(nc) as tc, ExitStack() as top:
        # ---- constants + persistent sbuf tiles -------------------------
        const = top.enter_context(tc.tile_pool(name="const", bufs=1))
        cos4 = const.tile([128, S], BF16, tag="cos4")
        sin4 = const.tile([128, S], BF16, tag="sin4")
        tri = const.tile([128, 128], BF16, tag="tri")
        id64 = const.tile([128, 64], BF16, tag="id64")
        ones_r = const.tile([1, 64], mybir.dt.float32r, tag="ones_r")
        # consts ride the scalar-engine HWDGE queue so they don't delay the
        # critical weights+x stream on the gpsimd SWDGE queue.
        nc.scalar.dma_start(id64[:], id_d[:])
        nc.scalar.dma_start(cos4[:], cos4_d[:])
        nc.scalar.dma_start(sin4[:], sin4_d[:])
        nc.scalar.dma_start(tri[:], tri_d[:])
        nc.gpsimd.dma_start(ones_r[:], ones64_d[:])

        wpool = top.enter_context(tc.tile_pool(name="w", bufs=1))
        wq_sb = wpool.tile([128, 8 * 256], BF16, tag="wq")
        wkv_sb = wpool.tile([128, 8 * 128], BF16, tag="wkv")
        wo_sb = wpool.tile([128, 2 * 1024], BF16, tag="wo")
        # qkv weights ride the sync HWDGE queue, in parallel with the x
        # chunks on the gpsimd SWDGE queue.
        nc.sync.dma_start(wkv_sb[:].rearrange("p (kc m) -> p kc m", kc=8),
                          wkv_ap.rearrange("(kc p) m -> p kc m", p=128))
        nc.sync.dma_start(wq_sb[:].rearrange("p (kc m) -> p kc m", kc=8),
                          wq_ap.rearrange("(kc p) m -> p kc m", p=128))

        # q'/k/v results live through the whole kernel
        act = top.enter_context(tc.tile_pool(name="acts", bufs=1))
        qp = [act.tile([128, S], BF16, tag=f"qp{i}", name=f"qp{i}")
              for i in range(2)]
        kk = act.tile([128, S], BF16, tag="kk")
        # v augmented with a ones column: PV drops the softmax denominator
        # into row 64 of each head's PSUM accumulator.
        v_all = act.tile([128, 16 * 80], BF16, tag="v_all")
        Y = [act.tile([128, S], BF16, tag=f"Y{i}", name=f"Y{i}")
             for i in range(2)]

        # ---- phase 1: projections, RoPE, v prep ------------------------
        with ExitStack() as ph1:
            raw = ph1.enter_context(tc.tile_pool(name="raw", bufs=1))
            qraw = [raw.tile([128, S], BF16, tag=f"qraw{i}", name=f"qraw{i}")
                    for i in range(2)]
            kvraw = raw.tile([128, S], BF16, tag="kvraw")
            qsh = [raw.tile([128, S], BF16, tag=f"qsh{i}", name=f"qsh{i}")
                   for i in range(2)]
            ksh = raw.tile([64, S], BF16, tag="ksh")

            phx = ph1.enter_context(ExitStack())
            xpool = phx.enter_context(tc.tile_pool(name="xT", bufs=1))
            xT = []
            for kc in range(8):
                t = xpool.tile([128, S], BF16, tag=f"x{kc}", name=f"x{kc}")
                nc.gpsimd.dma_start(t[:], xT_ap[kc * 128:(kc + 1) * 128, :])
                xT.append(t)
            # wo is not needed until the first out-projection (~mid-kernel);
            # issue it behind the x chunks.
            nc.gpsimd.dma_start(wo_sb[:].rearrange("p (c n) -> p c n", c=2),
                                wo_ap.rearrange("(c p) n -> p c n", p=128))

            pps = phx.enter_context(tc.tile_pool(name="pj", bufs=8,
                                                 space="PSUM"))
            with nc.named_scope("proj"):
                # kc-outer: each xT chunk is consumed right after its DMA
                # lands; kv + q-mt0 accumulate in pass 1 (8 banks), q-mt1
                # in pass 2.
                kv_ps = [pps.tile([128, 512], F32, tag="pj", name=f"kvps{nt}")
                         for nt in range(4)]
                q0_ps = [pps.tile([128, 512], F32, tag="pj", name=f"q0ps{nt}")
                         for nt in range(4)]
                for kc in range(8):
                    for nt in range(4):
                        nc.tensor.matmul(
                            kv_ps[nt][:], wkv_sb[:, kc * 128:(kc + 1) * 128],
                            xT[kc][:, nt * 512:(nt + 1) * 512],
                            start=(kc == 0), stop=(kc == 7))
                        nc.tensor.matmul(
                            q0_ps[nt][:], wq_sb[:, kc * 256:kc * 256 + 128],
                            xT[kc][:, nt * 512:(nt + 1) * 512],
                            start=(kc == 0), stop=(kc == 7))
                # kv copies on the scalar engine (idle in phase 1); they
                # gate the q1 bank reuse and the v transposes.
                for nt in range(4):
                    nc.scalar.copy(kvraw[:, nt * 512:(nt + 1) * 512],
                                   kv_ps[nt][:])
                q1_ps = [pps.tile([128, 512], F32, tag="pj", name=f"q1ps{nt}")
                         for nt in range(4)]
                for kc in range(8):
                    for nt in range(4):
                        nc.tensor.matmul(
                            q1_ps[nt][:], wq_sb[:, kc * 256 + 128:kc * 256 + 256],
                            xT[kc][:, nt * 512:(nt + 1) * 512],
                            start=(kc == 0), stop=(kc == 7))
                # q0 raw copies on DVE (run during pass 2, unblock rope-q0);
                # q1 raw copies on the scalar engine.
                for nt in range(4):
                    nc.vector.tensor_copy(qraw[0][:, nt * 512:(nt + 1) * 512],
                                          q0_ps[nt][:])
                for nt in range(4):
                    nc.scalar.copy(qraw[1][:, nt * 512:(nt + 1) * 512],
                                   q1_ps[nt][:])
            phx.close()

            def rope_q(i):
                for h in range(2):
                    b = h * 64
                    nc.vector.tensor_mul(qsh[i][b:b + 32, :],
                                         qraw[i][b + 32:b + 64, :],
                                         sin4[b + 32:b + 64, :])
                    nc.vector.tensor_mul(qsh[i][b + 32:b + 64, :],
                                         qraw[i][b:b + 32, :],
                                         sin4[b:b + 32, :])
                nc.vector.tensor_mul(qp[i][:], qraw[i][:], cos4[:])
                nc.vector.tensor_add(qp[i][:], qp[i][:], qsh[i][:])

            with nc.named_scope("rope"), ExitStack() as ph3:
                # rotate_half via partition-shifted DVE reads; the sin table
                # has its 32-row halves pre-swapped so both SBUF inputs share
                # a base partition (DVE requires equal SB-input bases; only
                # the output base may differ). DVE order: rope-k, rope-q
                # (pair 0), v copies, rope-q (pair 1) - matching when each
                # result is first consumed.
                nc.vector.tensor_mul(ksh[0:32, :], kvraw[32:64, :],
                                     sin4[32:64, :])
                nc.vector.tensor_mul(ksh[32:64, :], kvraw[0:32, :],
                                     sin4[0:32, :])
                nc.vector.tensor_mul(kk[0:64, :], kvraw[0:64, :], cos4[0:64, :])
                nc.vector.tensor_add(kk[0:64, :], kk[0:64, :], ksh[0:64, :])
                nc.vector.tensor_copy(kk[64:128, :], kk[0:64, :])
                rope_q(0)

                vps = ph3.enter_context(tc.tile_pool(name="vt", bufs=2,
                                                     space="PSUM"))
                ones_cols = v_all[:].rearrange("p (s c) -> p s c", c=80)[:, :, 64]
                nc.scalar.dma_start(ones_cols, ones16_d[:])
                for st in range(16):
                    tp = vps.tile([128, 64], BF16)
                    nc.tensor.transpose(
                        tp[:], kvraw[64:128, st * 128:(st + 1) * 128],
                        id64[64:128, :])
                    nc.vector.tensor_copy(v_all[:, st * 80:st * 80 + 64],
                                          tp[:])
                rope_q(1)

        # ---- attention + interleaved out-projection --------------------
        apool = top.enter_context(tc.tile_pool(name="at", bufs=2, space="PSUM"))
        epool = top.enter_context(tc.tile_pool(name="ex", bufs=3))
        dpool = top.enter_context(tc.tile_pool(name="div", bufs=2))
        osb = top.enter_context(tc.tile_pool(name="osb", bufs=4))
        oev = [0]

        def emit_outproj(qt):
            with nc.named_scope("outproj"):
                for st in range(4 * qt, 4 * qt + 4):
                    for nt in range(2):
                        po = apool.tile([128, 512], F32, tag="yo", bufs=4,
                                        name=f"po{st}{nt}")
                        for cc in range(2):
                            nc.tensor.matmul(
                                po[:],
                                Y[cc][:, st * 128:(st + 1) * 128],
                                wo_sb[:, cc * 1024 + nt * 512:
                                      cc * 1024 + (nt + 1) * 512],
                                start=(cc == 0), stop=(cc == 1))
                        ot = osb.tile([128, 512], BF16, tag="ot")
                        nc.vector.tensor_copy(ot[:], po[:])
                        oev[0] += 1
                        nc.sync.dma_start(
                            out_ap[st * 128:(st + 1) * 128,
                                   nt * 512:(nt + 1) * 512],
                            ot[:])

        with nc.named_scope("attn"):
            # descending qt: the dense q-tiles run first, keeping the PE's
            # HAM activity monitor at full clock through the early phase;
            # the sparse tail interleaves with out-projection filler.
            for qt in (3, 2, 1, 0):
                for pair in range(2):
                    nkc = 4 * qt + 4
                    yt = [apool.tile([128, 512], F32, tag="yo", bufs=4,
                                     name=f"y{pair}{qt}{_h}") for _h in range(2)]
                    for G in range(nkc // 2):
                        for hl in range(2):
                            hb = hl * 64
                            sc = apool.tile([128, 1024], F32, tag="sc", bufs=2,
                                            name=f"sc{pair}{qt}{G}{hl}")
                            regions = []
                            for ci in range(2):
                                kc = 2 * G + ci
                                j = kc - 4 * qt
                                off = j * 128 if j >= 0 else 0
                                trim = off if j >= 2 else 0
                                c0 = ci * 512
                                nc.tensor.matmul(
                                    sc[:, c0 + trim:c0 + 512],
                                    kk[hb:hb + 64, kc * 128:(kc + 1) * 128],
                                    qp[pair][hb:hb + 64,
                                             qt * 512 + trim:(qt + 1) * 512],
                                    start=True, stop=True)
                                regions.append((c0 + trim, c0 + 512))
                            ex = epool.tile([128, 1024], BF16, tag="ex",
                                            name=f"ex{pair}{qt}{G}{hl}")
                            if regions[0][0] == 0 and regions[1][0] == 512:
                                nc.scalar.activation(ex[:, 0:1024],
                                                     sc[:, 0:1024],
                                                     AF.Exp, scale=SCALE)
                            else:
                                for (a, b) in regions:
                                    nc.scalar.activation(ex[:, a:b],
                                                         sc[:, a:b],
                                                         AF.Exp, scale=SCALE)
                            # causal keep-mask on the diagonal 128-blocks
                            for ci in range(2):
                                kc = 2 * G + ci
                                j = kc - 4 * qt
                                if j >= 0:
                                    off = j * 128
                                    msl = ex[:, ci * 512 + off:
                                             ci * 512 + off + 128]
                                    nc.vector.tensor_mul(msl, msl, tri[:])
                            for ci in range(2):
                                kc = 2 * G + ci
                                j = kc - 4 * qt
                                off = j * 128 if j >= 0 else 0
                                nc.tensor.matmul(
                                    yt[hl][0:65, off:512],
                                    v_all[:, kc * 80:kc * 80 + 65],
                                    ex[:, ci * 512 + off:(ci + 1) * 512],
                                    start=(kc == 0), stop=(kc == nkc - 1))
                    # division (baseline flow): copy y+den to SBUF, recip,
                    # f32r cast, K=1 broadcast matmul, multiply, cast-DMA
                    # into the stacked bf16 Y tile.
                    ysb = dpool.tile([64, 1024], F32, tag="ysb")
                    dn = dpool.tile([1, 1024], F32, tag="dn")
                    nc.vector.tensor_copy(ysb[:, 0:512], yt[0][0:64, :])
                    nc.vector.tensor_copy(ysb[:, 512:1024], yt[1][0:64, :])
                    nc.scalar.copy(dn[:, 0:512], yt[0][64:65, :])
                    nc.scalar.copy(dn[:, 512:1024], yt[1][64:65, :])
                    recf = dpool.tile([1, 1024], F32, tag="recf")
                    nc.vector.reciprocal_approx_fast(recf[:], dn[:])
                    recr = dpool.tile([1, 1024], mybir.dt.float32r, tag="recr")
                    nc.vector.tensor_copy(recr[:], recf[:])
                    for hl in range(2):
                        bc_ps = apool.tile([128, 512], F32, tag="yo", bufs=4,
                                           name=f"bc{pair}{qt}{hl}")
                        nc.tensor.matmul(bc_ps[0:64, :], ones_r[:],
                                         recr[0:1, hl * 512:(hl + 1) * 512],
                                         start=True, stop=True)
                        # one PSUM operand + partition-shifted write into
                        # the stacked bf16 Y tile
                        nc.vector.tensor_mul(
                            Y[pair][hl * 64:hl * 64 + 64,
                                    qt * 512:(qt + 1) * 512],
                            ysb[:, hl * 512:(hl + 1) * 512], bc_ps[0:64, :])
                    # out-projection of the previously processed q-tile,
                    # emitted here so the PE fills the gap while this
                    # q-tile's exps drain.
                    if pair == 0 and qt <= 2:
                        emit_outproj(qt + 1)
            emit_outproj(0)

    nc.compile()
    return nc


def _shard_inputs(x, Wq, Wk, Wv, Wo):
    bf = ml_dtypes.bfloat16
    in_maps = []
    for c in range(N_CORES):
        b, g = divmod(c, 4)
        in_maps.append({
            "xT": np.ascontiguousarray(x[b].T).astype(bf),
            "wq": np.ascontiguousarray(
                Wq[:, g * 256:(g + 1) * 256]).astype(bf),
            "wkv": np.ascontiguousarray(np.concatenate(
                [Wk[:, g * 64:(g + 1) * 64], Wv[:, g * 64:(g + 1) * 64]],
                axis=1)).astype(bf),
            "wo": np.ascontiguousarray(
                Wo[g * 256:(g + 1) * 256, :]).astype(bf),
        })
    return in_maps


def kernel(x, Wq, Wk, Wv, Wo):
    x = np.asarray(x, dtype=np.float32)
    Wq = np.asarray(Wq, dtype=np.float32)
    Wk = np.asarray(Wk, dtype=np.float32)
    Wv = np.asarray(Wv, dtype=np.float32)
    Wo = np.asarray(Wo, dtype=np.float32)
    assert x.shape == (B, S, D), x.shape

    if "nc" not in _CACHE:
        _CACHE["nc"] = _build_kernel()
    nc = _CACHE["nc"]

    in_maps = _shard_inputs(x, Wq, Wk, Wv, Wo)
    res = run_bass_kernel_spmd(nc, in_maps, list(range(N_CORES)))

    out = np.zeros((B, S, D), dtype=np.float32)
    for c in range(N_CORES):
        out[c // 4] += np.asarray(res.results[c]["out"]).astype(np.float32)
    return out
